# revision 55
# baseline (speedup 1.0000x reference)
"""Trainium2 Bass kernel for IR-Net style binarized conv block.

Computation (matches the reference nn.Module):
  1. Per-out-channel weight standardization -> sign -> {-1,+1}, power-of-2
     per-channel scale sw (host-side numpy; weights are tiny).
  2. ba = sign(x) (device, ScalarE Sign activation, exact in fp8).
  3. y = conv2d(ba, sign_w) * sw  -- 3x3, pad 1, stride 1. Done as 9 shifted
     matmuls over a zero-padded SBUF activation image, channels on the
     partition axis, accumulating in PSUM. Exact: products are +-1 summed in
     fp32 PSUM.
  4. Training-mode BatchNorm over the FULL batch: per-channel sum / sumsq are
     accumulated on-device (activation accum_out / tensor_scalar accum_out),
     AllGather'd across the 8 cores (2KB), folded together with sw, gamma,
     beta into per-channel affine a*z + b.
  5. Hardtanh clip via tensor_scalar(min,max).

Sharding: pure data parallel, batch 32 -> 4 images per core x 8 cores.

Pipeline (v2):
  - input DMAs are paced one image ahead of the conv and alternate between
    the two HWDGE rings (sync / scalar) so they don't serialize behind each
    other on one ring.
  - group 0's BN-apply (affine on ScalarE, clip on VectorE, out-DMA on sync)
    is interleaved between the tail tiles of group 1's conv so it hides
    under the matmuls.
  - group 1's apply is the only exposed tail: stats AllGather + affine
    split across ScalarE/VectorE + out-DMAs alternating both rings.
"""

import numpy as np
import ml_dtypes

import concourse.bacc as bacc
import concourse.bass as bass
import concourse.tile as tile
from concourse import mybir
from concourse.bass_utils import run_bass_kernel_spmd

F32 = mybir.dt.float32
BF16 = mybir.dt.bfloat16
FP8 = mybir.dt.float8e4

P = 128          # SBUF partitions
CG = 2           # channel groups: 256 channels = 2 x 128
C = 256
BN_EPS = 1e-5
N_CORES = 8
WPAD = 60        # padded row length (w+2 <= 60; 58 measured slower per-MM)
RT = 8           # output rows per PSUM tile (8 * 60 = 480 <= 512 fp32/bank)


def build_kernel(b_per_core=4, h=56, w=56, n_cores=N_CORES, use_fp8=True):
    """Build the per-core Bass program. Returns the compiled Bacc instance."""
    # fp8 DoubleRow needs a flat 3D moving AP [K, 2, N]; row-blocks are taken
    # as contiguous RT*WPAD slices of the padded image, which can overrun the
    # last padded row by up to kh*WPAD+kw -- give each image 2 spare zero
    # rows at the bottom.
    HP = h + 2 + (2 if use_fp8 else 0)
    assert w + 2 <= WPAD
    assert h % RT == 0
    tiles_per_img = h // RT
    NT = b_per_core * tiles_per_img     # PSUM tiles per output-channel group
    FREE = RT * w                       # useful free dim per tile (compact)
    MMF = RT * WPAD                     # moving free dim per fp8 matmul
    nhw_total = n_cores * b_per_core * h * w
    adt = FP8 if use_fp8 else BF16
    # fp8 DoubleRow: the CG-dim stride of the moving AP must be 16B aligned
    assert not use_fp8 or (b_per_core * HP * WPAD) % 16 == 0

    nc = bacc.Bacc(
        "TRN2", target_bir_lowering=False, debug=False, num_devices=n_cores
    )
    x_d = nc.dram_tensor("x", [b_per_core, C, h, w], F32, kind="ExternalInput").ap()
    w_d = nc.dram_tensor("wsgn", [P, CG, 9, C], adt, kind="ExternalInput").ap()
    coef_d = nc.dram_tensor("coef", [P, CG, 3], F32, kind="ExternalInput").ap()
    out_d = nc.dram_tensor(
        "out", [b_per_core, C, h, w], F32, kind="ExternalOutput"
    ).ap()

    mult = mybir.AluOpType.mult
    add = mybir.AluOpType.add
    subtract = mybir.AluOpType.subtract
    amin = mybir.AluOpType.min
    amax = mybir.AluOpType.max
    AF = mybir.ActivationFunctionType

    with tile.TileContext(nc) as tc:
        with (
            tc.tile_pool(name="singles", bufs=1) as singles,
            tc.tile_pool(name="xs", bufs=8) as xs_pool,
            tc.tile_pool(name="psum", bufs=7, space="PSUM") as psum_pool,
            tc.tile_pool(name="psumw", bufs=1, space="PSUM") as psumw_pool,
            tc.tile_pool(name="sq", bufs=2) as sq_pool,
            tc.tile_pool(name="small", bufs=1) as small,
            tc.tile_pool(name="dram", bufs=1, space="DRAM") as dram,
        ):
            # ---- constants ----
            # weights laid out [P, out-group, in-group, tap, 128] so each
            # output-channel group is one contiguous DMA; group 0's first
            # LDWEIGHTS doesn't wait for the whole tensor
            wsb = singles.tile([P, CG, CG, 9, P], adt)
            coef = singles.tile([P, CG, 3], F32)
            nc.scalar.dma_start(out=coef[:], in_=coef_d)

            # scratch for PE warm-up matmuls (see below)
            scratch = small.tile([P, 512], adt)
            nc.gpsimd.memset(scratch[:], 0.0)

            # ---- padded, binarized activations (resident) ----
            # only padded row 0 / row h+1 and cols 0 / w+1 are read by real
            # (kept) outputs; the fp8 overrun rows h+2.. only feed dropped
            # columns, so they can stay uninitialized. Memsets go on the
            # otherwise-idle gpsimd queue so the first matmul (which reads
            # the borders) isn't gated on the busy vector queue.
            acts = singles.tile([P, CG, b_per_core, HP, WPAD], adt)
            nc.gpsimd.memset(acts[:, :, :, 0, :], 0.0)
            nc.gpsimd.memset(acts[:, :, :, h + 1, :], 0.0)
            nc.gpsimd.memset(acts[:, :, :, 0 : h + 2, 0:1], 0.0)
            nc.gpsimd.memset(acts[:, :, :, 0 : h + 2, w + 1 : w + 2], 0.0)

            # ---- PE warm-up: ~6us of dummy matmuls so the HAM clock-gate
            # releases (1.2 -> 2.4 GHz) before the first real matmul, which
            # can only start once the first x slices are DMA'd + binarized.
            warm_ps = psumw_pool.tile([P, MMF], F32, tag="warm")
            for _ in range(14):
                nc.tensor.matmul(
                    warm_ps[:],
                    lhsT=scratch[:, 0:P],
                    rhs=scratch[:, 0:MMF],
                    start=True,
                    stop=True,
                )

            # x is DMA'd per (image, channel-group, row-quarter) and
            # binarized with ACT Sign. DMAs alternate between the two HWDGE
            # rings (sync / scalar); a single ring drains serially and the
            # gpsimd SWDGE path is too slow to keep the conv fed.
            NQ = 4
            hh = h // NQ
            dma_rr = [0]

            def emit_bin_pair(n, q, a):
                xt = xs_pool.tile([P, hh, w], F32, tag="xstage")
                eng = nc.sync if (dma_rr[0] % 2 == 0) else nc.scalar
                dma_rr[0] += 1
                eng.dma_start(
                    out=xt[:],
                    in_=x_d[n, a * P : (a + 1) * P, q * hh : (q + 1) * hh, :],
                )
                nc.scalar.activation(
                    out=acts[:, a, n, 1 + q * hh : 1 + (q + 1) * hh, 1 : w + 1],
                    in_=xt[:],
                    func=AF.Sign,
                )

            # image 0 upfront; the rest paced inside the group-0 conv loop
            bin_queue = [
                (n, q, a)
                for n in range(b_per_core)
                for q in range(NQ)
                for a in range(CG)
            ]
            pairs_per_img = NQ * CG
            total_pairs = len(bin_queue)
            # rows needed by the first couple of conv tiles go out first;
            # the (small) weight DMAs slot in behind them on the rings.
            # image 1's first half also goes out upfront (the xs pool depth
            # paces it). NOTE: queueing much more than this upfront makes
            # things WORSE -- concurrent DMAs on a ring share bandwidth, so
            # a flood makes every transfer (including image 0's) crawl.
            for _ in range(4):
                emit_bin_pair(*bin_queue.pop(0))
            nc.sync.dma_start(out=wsb[:, 0], in_=w_d[:, 0])
            for _ in range(pairs_per_img - 4):
                emit_bin_pair(*bin_queue.pop(0))
            # second half of the weights after image 0's input is queued
            nc.sync.dma_start(out=wsb[:, 1], in_=w_d[:, 1])
            for _ in range(4):
                emit_bin_pair(*bin_queue.pop(0))

            # ---- warm-up AllGather: aligns the 8 cores early (they start
            # with ~10us of skew) so the stats collectives later don't pay
            # the skew as trigger-wait; also warms the collective firmware.
            # Runs on the CC cores / SDMA, fully overlapped with conv.
            wu_src = small.tile([P, 2], F32)
            nc.gpsimd.memset(wu_src[:], 0.0)
            wu_in = dram.tile([P, 2], F32, tag="wuin")
            wu_out = dram.tile([n_cores * P, 2], F32, tag="wuout")
            nc.gpsimd.dma_start(out=wu_in[:], in_=wu_src[:])
            nc.gpsimd.collective_compute(
                "AllGather",
                mybir.AluOpType.bypass,
                replica_groups=[list(range(n_cores))],
                ins=[wu_in.opt()],
                outs=[wu_out.opt()],
            )

            # ---- conv + BN, pipelined per output-channel group ----
            # separate tiles per group so group 0's apply (reads ybufs[0])
            # carries no dependency on group 1's evictions (write ybufs[1])
            ybufs = [
                singles.tile([P, NT, FREE], F32, tag=f"ybuf{b}", name=f"ybuf{b}")
                for b in range(CG)
            ]
            sum_ps = [
                small.tile([P, NT], F32, tag=f"sum{b}", name=f"sum{b}") for b in range(CG)
            ]
            sumsq_ps = [
                small.tile([P, NT], F32, tag=f"sumsq{b}", name=f"sumsq{b}") for b in range(CG)
            ]
            eps_t = small.tile([P, 1], F32)
            nc.vector.memset(eps_t[:], BN_EPS)
            # preload the Sqrt ACT table now so the coef chain later doesn't
            # pay the ~1.3us ACT_TABLE_LOAD on its critical path
            sq_warm = small.tile([P, 1], F32)
            nc.scalar.activation(out=sq_warm[:], in_=eps_t[:], func=AF.Sqrt)
            CH = 2
            cfs = [None, None]  # per-group affine coefficient tiles

            def emit_conv_tile(b, n, t):
                r0 = t * RT
                if use_fp8:
                    # DoubleRow: contract both input channel groups at once.
                    # Moving AP must be flat 3D [K, 2, N]: contiguous
                    # RT*WPAD row-blocks (2 garbage cols per row, dropped
                    # at eviction).
                    ps = psum_pool.tile([P, MMF], F32, tag="ps")
                    flat = acts[:, :, n, :, :].rearrange("p g h w -> p g (h w)")
                    k = 0
                    for kh in range(3):
                        for kw in range(3):
                            st = (r0 + kh) * WPAD + kw
                            nc.tensor.matmul(
                                ps[:],
                                lhsT=wsb[:, b, :, kh * 3 + kw, :],
                                rhs=flat[:, :, st : st + MMF],
                                start=(k == 0),
                                stop=(k == 8),
                                perf_mode=mybir.MatmulPerfMode.DoubleRow,
                            )
                            k += 1
                    ps_v = ps[:].rearrange("p (r c) -> p r c", r=RT)[:, :, 0:w]
                else:
                    ps = psum_pool.tile([P, FREE], F32, tag="ps")
                    k = 0
                    for a in range(CG):
                        for kh in range(3):
                            for kw in range(3):
                                nc.tensor.matmul(
                                    ps[:],
                                    lhsT=wsb[:, b, a, kh * 3 + kw, :],
                                    rhs=acts[
                                        :, a, n, r0 + kh : r0 + kh + RT,
                                        kw : kw + w,
                                    ],
                                    start=(k == 0),
                                    stop=(k == 17),
                                )
                                k += 1
                    ps_v = ps[:]
                idx = n * tiles_per_img + t
                # evict: copy PSUM->SBUF + per-channel sum (VectorE; single
                # PSUM input -- PSUM has one DVE read port)
                nc.vector.tensor_scalar(
                    out=ybufs[b][:, idx, :],
                    in0=ps_v,
                    scalar1=0.0,
                    scalar2=None,
                    op0=add,
                    op1=add,
                    accum_out=sum_ps[b][:, idx : idx + 1],
                )
                # square + per-channel sumsq (ScalarE)
                sqt = sq_pool.tile([P, FREE], F32, tag="sq")
                nc.scalar.activation(
                    out=sqt[:],
                    in_=ps_v,
                    func=AF.Square,
                    accum_out=sumsq_ps[b][:, idx : idx + 1],
                )

            gst8s = [None, None]

            def emit_stats(b, reduce_eng, dma_eng):
                """Local stats reduce + AllGather trigger (critical chain).

                Group 0's chain runs mid-conv, so it uses the idle gpsimd
                queue (Vector/Scalar are busy with evictions/Squares and
                would sit on it until conv end). Group 1's runs at conv end
                when Vector/Scalar free up immediately."""
                stats_b = small.tile([P, 2], F32, tag=f"stats{b}")
                reduce_eng.tensor_reduce(
                    out=stats_b[:, 0:1], in_=sum_ps[b][:],
                    axis=mybir.AxisListType.X, op=add,
                )
                reduce_eng.tensor_reduce(
                    out=stats_b[:, 1:2], in_=sumsq_ps[b][:],
                    axis=mybir.AxisListType.X, op=add,
                )
                in_bounce = dram.tile([P, 2], F32, tag=f"inb{b}")
                out_bounce = dram.tile([n_cores * P, 2], F32, tag=f"outb{b}")
                dma_eng.dma_start(out=in_bounce[:], in_=stats_b[:])
                nc.gpsimd.collective_compute(
                    "AllGather",
                    mybir.AluOpType.bypass,
                    replica_groups=[list(range(n_cores))],
                    ins=[in_bounce.opt()],
                    outs=[out_bounce.opt()],
                )
                return out_bounce

            # (emit_gather defined below emits the gather-back DMA)

            def emit_gather(b, out_bounce, eng):
                """DMA the gathered stats back. The trigger blocks its engine
                queue (and its HWDGE ring) until the collective completes, so
                the caller picks an engine/position where that's harmless."""
                gst8 = small.tile([P, 2, n_cores], F32, tag=f"gst8{b}")
                eng.dma_start(
                    out=gst8[:],
                    in_=out_bounce[:].rearrange("(c p) s -> p s c", c=n_cores),
                )
                gst8s[b] = gst8

            def emit_coef(b):
                gstats = small.tile([P, 2], F32, tag=f"gstats{b}")
                nc.vector.tensor_reduce(
                    out=gstats[:], in_=gst8s[b][:],
                    axis=mybir.AxisListType.X, op=add,
                )

                # ---- per-channel affine coefficients for this group ----
                # mean = sum/nhw; ex2 = sumsq/nhw; var_y = (ex2-mean^2)*sw^2
                # rstd = 1/sqrt(var_y+eps); a = gamma*sw*rstd; b = beta-mean*a
                cf = small.tile([P, 6], F32, tag=f"cf{b}")
                mean_t, ex2_t, var_t, std_t, a_t, b_t = (
                    cf[:, i : i + 1] for i in range(6)
                )
                nc.vector.tensor_scalar_mul(mean_t, gstats[:, 0:1], 1.0 / nhw_total)
                nc.vector.tensor_scalar_mul(ex2_t, gstats[:, 1:2], 1.0 / nhw_total)
                # var = (ex2 - mean*mean) * sw2, via (mean*mean - ex2) * -sw2
                nc.vector.scalar_tensor_tensor(
                    out=var_t, in0=mean_t, scalar=mean_t, in1=ex2_t,
                    op0=mult, op1=subtract,
                )
                nc.vector.tensor_tensor(
                    out=var_t, in0=var_t, in1=coef[:, b, 2:3], op=mult
                )
                nc.vector.tensor_scalar_mul(var_t, var_t, -1.0)
                nc.scalar.activation(
                    out=std_t, in_=var_t, func=AF.Sqrt, bias=eps_t[:], scale=1.0
                )
                nc.vector.reciprocal(out=std_t, in_=std_t)
                nc.vector.tensor_tensor(
                    out=a_t, in0=coef[:, b, 0:1], in1=std_t, op=mult
                )
                nc.vector.scalar_tensor_tensor(
                    out=b_t, in0=mean_t, scalar=-1.0, in1=a_t, op0=mult, op1=mult
                )
                nc.vector.tensor_tensor(
                    out=b_t, in0=coef[:, b, 1:2], in1=b_t, op=add
                )
                cfs[b] = cf

            def emit_apply_chunk(b, chunk, affine_scalar, dma_eng):
                """chunk = (n, t0, ch): affine+hardtanh+store for ch tiles.

                Works in-place in ybuf (y isn't needed afterwards), so the
                apply has NO buffer-rotation WAR deps: its only waits are on
                the coefficients and on ybuf itself. In particular it never
                waits on DMA-completion semaphores, which stall while a
                collective is in flight."""
                n, t0, ch = chunk
                cf = cfs[b]
                a_t, b_t = cf[:, 4:5], cf[:, 5:6]
                idx = n * tiles_per_img + t0
                yv = ybufs[b][:, idx : idx + ch, :]
                if affine_scalar:
                    nc.scalar.activation(
                        out=yv, in_=yv, func=AF.Identity, bias=b_t, scale=a_t,
                    )
                else:
                    nc.vector.tensor_scalar(
                        out=yv, in0=yv,
                        scalar1=a_t, scalar2=b_t, op0=mult, op1=add,
                    )
                nc.vector.tensor_scalar(
                    out=yv, in0=yv,
                    scalar1=1.0, scalar2=-1.0, op0=amin, op1=amax,
                )
                dma_eng.dma_start(
                    out=out_d[n, b * P : (b + 1) * P, t0 * RT : (t0 + ch) * RT, :],
                    in_=yv,
                )

            def chunks_of_group():
                out = []
                for n in range(b_per_core):
                    t = 0
                    while t < tiles_per_img:
                        ch = min(CH, tiles_per_img - t)
                        out.append((n, t, ch))
                        t += ch
                return out

            chunks = chunks_of_group()  # 16 chunks per group (2+2+2+1/img)

            # ---- group 0 conv; binarize paced one image ahead ----
            # during image n's tiles, emit image n+1's (dma, sign) pairs so
            # each image is fully binarized one image before it's consumed
            # Conv, interleaved by image: g0-n then g1-n. Group 1 reuses
            # image n's binarized acts, so each image's x stream gets two
            # images' worth of conv time -- without this, group 0's conv at
            # warm PE speed demands ~480GB/s of input vs 358GB/s of HBM and
            # stalls ~10us at image transitions.
            def pace_to(target):
                t2 = min(total_pairs, target)
                while (total_pairs - len(bin_queue)) < t2 and bin_queue:
                    emit_bin_pair(*bin_queue.pop(0))

            for n in range(b_per_core):
                for t in range(tiles_per_img):
                    emit_conv_tile(0, n, t)
                    # ~1 pair of image n+1 per g0 tile
                    pace_to(pairs_per_img * (n + 1) + (t + 1))
                if n == b_per_core - 1:
                    # all group-0 tiles done: fire its stats collective now;
                    # it pipelines ahead of group 1's on the CC engine
                    ob0 = emit_stats(0, nc.vector, nc.gpsimd)
                    emit_gather(0, ob0, nc.sync)
                    emit_coef(0)
                for t in range(tiles_per_img):
                    emit_conv_tile(1, n, t)
                    # the rest of image n+1 early in g1-n's tiles
                    pace_to(pairs_per_img * (n + 1) + 7 + (t + 1) * 2)
            while bin_queue:
                emit_bin_pair(*bin_queue.pop(0))

            # ---- tail ----
            # 1. group 1's stats chain first: it's the critical path
            #    (reduce -> bounce -> AllGather across cores).
            ob1 = emit_stats(1, nc.vector, nc.scalar)
            # 2. group 0's apply runs while that AllGather is in flight
            #    (its coefficients have long been ready). Affines split
            #    ScalarE/VectorE, clips on VectorE; stores alternate between
            #    the sync HWDGE ring and the gpsimd SWDGE path so the
            #    ScalarE queue carries no DMA triggers and no single ring
            #    serializes the ~3.7MB of stores.
            for j, chunk in enumerate(chunks):
                emit_apply_chunk(
                    0,
                    chunk,
                    affine_scalar=(j % 3 != 2),  # ~11 ScalarE / 5 VectorE
                    dma_eng=nc.sync,
                )
            # 3. gather the stats back (scalar ring + ScalarE queue position
            #    after the apply-0 affines, so its collective-wait sits
            #    behind them) and finish group 1.
            emit_gather(1, ob1, nc.scalar)
            emit_coef(1)
            # ScalarE also carries the odd store triggers here, so it gets
            # only half the affines (8x affine + 8x trigger ~= VectorE's
            # 8x affine + 16x clip)
            for j, chunk in enumerate(chunks):
                emit_apply_chunk(
                    1,
                    chunk,
                    affine_scalar=(j % 2 == 0),
                    dma_eng=nc.sync if j % 2 == 0 else nc.scalar,
                )

    nc.compile()
    return nc


def prep_inputs(x, weight, gamma, beta, b_per_core, n_cores, use_fp8=True):
    """Host-side prep: weight standardization/sign/scale + sharding."""
    w64 = np.asarray(weight, dtype=np.float64)
    co = w64.shape[0]
    wf = w64.reshape(co, -1)
    mean = wf.mean(axis=1)
    bw = w64 - mean[:, None, None, None]
    std = bw.reshape(co, -1).std(axis=1, ddof=1)
    mb = np.abs(bw / std[:, None, None, None]).reshape(co, -1).mean(axis=1)
    sw = 2.0 ** np.round(np.log2(mb))
    sgn = np.sign(bw)  # {-1, 0, +1}

    # wsgn[p, b, a, t, co128] = sgn[b*128+co128, a*128+p, kh, kw]
    s = sgn.reshape(CG, P, CG, P, 9)  # [b, co128, a, p, t]
    wsgn = np.ascontiguousarray(s.transpose(3, 0, 2, 4, 1))
    adt_np = ml_dtypes.float8_e4m3 if use_fp8 else ml_dtypes.bfloat16
    wsgn = wsgn.astype(adt_np)

    ga = (np.asarray(gamma, dtype=np.float64) * sw).astype(np.float32)
    be = np.asarray(beta, dtype=np.float32)
    sw2 = (sw * sw).astype(np.float32)
    coef = np.stack(
        [
            ga.reshape(CG, P).T,       # [p, g]
            be.reshape(CG, P).T,
            sw2.reshape(CG, P).T,
        ],
        axis=-1,
    ).astype(np.float32)               # [P, CG, 3]

    x = np.asarray(x, dtype=np.float32)
    in_maps = []
    for c in range(n_cores):
        in_maps.append(
            {
                "x": np.ascontiguousarray(
                    x[c * b_per_core : (c + 1) * b_per_core]
                ),
                "wsgn": wsgn,
                "coef": coef,
            }
        )
    return in_maps


_CACHE = {}


def _get_nc(key, **kw):
    if key not in _CACHE:
        _CACHE[key] = build_kernel(**kw)
    return _CACHE[key]


def run(x, weight, gamma, beta, use_fp8=True, trace=False):
    n, c, h, w = x.shape
    b_per_core = n // N_CORES
    nc = _get_nc(
        (b_per_core, h, w, use_fp8),
        b_per_core=b_per_core,
        h=h,
        w=w,
        n_cores=N_CORES,
        use_fp8=use_fp8,
    )
    in_maps = prep_inputs(
        x, weight, gamma, beta, b_per_core, N_CORES, use_fp8=use_fp8
    )
    import os
    tkw = {}
    if os.environ.get("ALLCORES"):
        tkw["trace_cores"] = list(range(N_CORES))
    res = run_bass_kernel_spmd(nc, in_maps, list(range(N_CORES)), trace=trace, **tkw)
    out = np.concatenate([r["out"] for r in res.results], axis=0)
    return out, res


def kernel(x, weight, gamma, beta):
    out, _ = run(x, weight, gamma, beta, use_fp8=True)
    return out
